# revision 7
# baseline (speedup 1.0000x reference)
"""Trainium2 Bass kernel for nn_BilinearDecoder.

Math (per cell c, pair p):
    out[c,p] = sum_{n,k} emb[i0,n] * wl[c,n] * W[n,k] * wl[c,k] * emb[i1,k]

Restructured as:
    That_c[e,n] = wl[c,n] * sum_k W[n,k] * wl[c,k] * emb[e,k]   (matmul over entities)
    out[c,p]   = sum_n emb[i0[c,p],n] * That_c[i1[c,p],n]       (gather + dot)

Sharding: data-parallel over cells. 39 cells -> 8 cores x 5 slots (last slot
of core 7 is padding). Embedding + weights replicated per core.

Per-core pipeline (Tile framework):
  - load embT (fp32, lhsT for matmul), W^T, wl, wrapped int16 gather indices
  - per cell: build W_cT = W^T * wl[k](partition) * wl[n](free)  (vector)
              That = emb @ W_cT  (128 matmuls, fp32, PSUM accumulate)
              cast That -> fp16, store to DRAM scratch
              dma_gather rows of emb16 (A side) and That (B side), fp16
              tensor_mul + tensor_reduce -> out columns

dma_gather layout contracts (HW-validated):
  - indices int16, SBUF tile [128, n/16]: idx j at [j%16, j//16], the 16-row
    pattern replicated 8x down the partitions.
  - output [128, n/128, D]: row j lands at partition j%128, free tile j//128.
Output pair t*128+p therefore sits at out partition p, column t; the host
transposes [CLOC, 128, 64] -> [CLOC, 8192].
"""

import numpy as np
from contextlib import ExitStack

import concourse.bass as bass
import concourse.tile as tile
from concourse import bacc, mybir
from concourse.bass_utils import run_bass_kernel_spmd
from bass_rust import add_dep_helper

CELLS, PAIRS, D, N = 39, 8192, 512, 4096
NCORES, CLOC = 8, 5  # 8 cores x 5 cell slots = 40 >= 39
NCHUNK = 8  # gather chunks per cell
CHUNK = PAIRS // NCHUNK  # 512 pairs per chunk
NSUB = CHUNK // 128  # 4 sub-tiles of 128 pairs per chunk

F32 = mybir.dt.float32
F16 = mybir.dt.float16
I16 = mybir.dt.int16

_PROGRAM = None


def build_program(nchunk=8, gp_bufs=4, zp_bufs=2, tst_bufs=3, psum_bufs=6,
                  op_bufs=CLOC, ag_bufs=7, do_matmul=True, do_gather=True,
                  do_dot=True, reps=1, nqueues=4):
    chunk = PAIRS // nchunk
    nsub = chunk // 128
    nc = bacc.Bacc("TRN2", target_bir_lowering=False, debug=False,
                   num_swdge_queues=nqueues)

    embT = nc.dram_tensor("embT", [D, N], F16, kind="ExternalInput")
    emb16 = nc.dram_tensor("emb16", [N, D], F16, kind="ExternalInput")
    wt = nc.dram_tensor("wt", [D, D], F32, kind="ExternalInput")
    wl = nc.dram_tensor("wl", [CLOC, D], F32, kind="ExternalInput")
    wlcol = nc.dram_tensor("wlcol", [128, CLOC, 4], F32, kind="ExternalInput")
    idx = nc.dram_tensor("idx", [128, 2, CLOC, PAIRS // 16], I16,
                         kind="ExternalInput")
    outv = nc.dram_tensor("outv", [CLOC, 128, PAIRS // 128], F32,
                          kind="ExternalOutput")
    that = nc.dram_tensor("that", [CLOC, N, D], F16)  # internal scratch

    with tile.TileContext(nc) as tc, ExitStack() as ctx:
        singles = ctx.enter_context(tc.tile_pool(name="singles", bufs=1))
        idxp = ctx.enter_context(tc.tile_pool(name="idxp", bufs=2))
        wlp = ctx.enter_context(tc.tile_pool(name="wlp", bufs=2))
        psum = ctx.enter_context(tc.tile_pool(name="psum", bufs=psum_bufs, space="PSUM"))
        zp = ctx.enter_context(tc.tile_pool(name="zp", bufs=zp_bufs))
        tstp = ctx.enter_context(tc.tile_pool(name="tst", bufs=tst_bufs))
        gp = ctx.enter_context(tc.tile_pool(name="gp", bufs=gp_bufs))
        op = ctx.enter_context(tc.tile_pool(name="op", bufs=op_bufs))
        agp = ctx.enter_context(tc.tile_pool(name="agp", bufs=ag_bufs))

        # The rep loop replays the COMPLETE kernel body (input loads, weight
        # prep, compute, output stores) so that test.py's replication
        # differencing measures one full execution, not just the inner loop.
        for rep in range(reps):
            # ---- per-execution loads ----
            embT_t = singles.tile([128, 4, N], F16, tag="embT")
            for kt in range(4):
                nc.sync.dma_start(embT_t[:, kt, :], embT[kt * 128:(kt + 1) * 128, :])
            wt_t = singles.tile([128, 4, D], F32, tag="wt")
            for kt in range(4):
                nc.sync.dma_start(wt_t[:, kt, :], wt[kt * 128:(kt + 1) * 128, :])
            wlcol_t = singles.tile([128, CLOC, 4], F32, tag="wlcol")
            nc.sync.dma_start(wlcol_t, wlcol[:])
            # double-buffered: the reload for rep r+1 must not wait for all of
            # rep r's gathers to release the tile
            idx_t = idxp.tile([128, 2, CLOC, PAIRS // 16], I16, tag="idx")
            nc.sync.dma_start(idx_t, idx[:])

            # ---- build all W_cT up-front (keeps DVE free during dot stage) ----
            wct_t = singles.tile([128, CLOC, 4, D], F16, tag="wct")
            for c in range(CLOC):
                wlr = wlp.tile([128, D], F32)
                nc.scalar.dma_start(wlr, wl[c:c + 1, :].to_broadcast([128, D]))
                for kt in range(4):
                    nc.vector.tensor_mul(wct_t[:, c, kt], wt_t[:, kt], wlr)
                    nc.vector.tensor_scalar_mul(
                        wct_t[:, c, kt], wct_t[:, c, kt], wlcol_t[:, c, kt:kt + 1]
                    )

            def issue_ags(c):
                ags = []
                for ch in range(nchunk if do_gather else 0):
                    ag = agp.tile([128, nsub, D], F16, tag="ag")
                    isl = slice(ch * (chunk // 16), (ch + 1) * (chunk // 16))
                    nc.gpsimd.dma_gather(
                        ag,
                        emb16[:],
                        idx_t[:, 0, c, isl],
                        num_idxs=chunk,
                        num_idxs_reg=chunk,
                        elem_size=D,
                        queue_num=ch % nqueues,
                    )
                    ags.append(ag)
                return ags

            # A-side gathers are issued one cell ahead of the That-side
            # gathers: tg waits on the cell's That stores and would otherwise
            # head-of-line block the in-order Pool engine, starving the DMA
            # rings while the matmul phase finishes.
            ags_pending = issue_ags(0)
            outsbs = []
            for c in range(CLOC):
                that_stores = []
                # ---- That_c = emb @ W_cT : [N, D] fp32 -> fp16 -> DRAM ----
                # 4 et-tiles share one store (bigger DMAs, fewer ops contending
                # with the gather stream)
                tst = None
                for et in range(N // 128 if do_matmul else 0):
                    ps = psum.tile([128, D], F32)
                    for kt in range(4):
                        nc.tensor.matmul(
                            ps,
                            embT_t[:, kt, et * 128:(et + 1) * 128],
                            wct_t[:, c, kt],
                            start=(kt == 0),
                            stop=(kt == 3),
                        )
                    if et % 4 == 0:
                        tst = tstp.tile([128, 4, D], F16)
                    nc.scalar.copy(tst[:, et % 4, :], ps)
                    if et % 4 == 3:
                        st_inst = nc.sync.dma_start(
                            that[c, (et - 3) * 128:(et + 1) * 128, :].rearrange(
                                "(j p) d -> p j d", p=128
                            ),
                            tst,
                        )
                        that_stores.append(st_inst)

                # ---- gather + dot ----
                ags = ags_pending
                if c + 1 < CLOC:
                    ags_pending = issue_ags(c + 1)
                outsb = op.tile([128, PAIRS // 128], F32)
                if not do_dot:
                    nc.vector.memset(outsb, 0.0)
                for ch in range(nchunk if do_gather else 0):
                    tg = gp.tile([128, nsub, D], F16, tag="tg")
                    isl = slice(ch * (chunk // 16), (ch + 1) * (chunk // 16))
                    tg_inst = nc.gpsimd.dma_gather(
                        tg,
                        that[c] if do_matmul else emb16[:],
                        idx_t[:, 1, c, isl],
                        num_idxs=chunk,
                        num_idxs_reg=chunk,
                        elem_size=D,
                        queue_num=ch % nqueues,
                    )
                    for st_inst in that_stores:
                        add_dep_helper(tg_inst.ins, st_inst.ins,
                                       reason="that DRAM write -> gather read")
                    if do_dot:
                        z = zp.tile([128, nsub, D], F16)
                        nc.vector.tensor_mul(z, ags[ch], tg)
                        nc.vector.tensor_reduce(
                            outsb[:, ch * nsub:(ch + 1) * nsub],
                            z,
                            axis=mybir.AxisListType.X,
                            op=mybir.AluOpType.add,
                        )
                outsbs.append(outsb)

            # outv stores issue from the Activation HWDGE queue: on the SP
            # queue they would head-of-line block the next rep's loads (they
            # wait on the full dot phase)
            for c in range(CLOC):
                nc.scalar.dma_start(outv[c], outsbs[-CLOC + c])

    nc.compile()
    return nc


def get_program():
    global _PROGRAM
    if _PROGRAM is None:
        _PROGRAM = build_program()
    return _PROGRAM


def make_in_maps(embedding, index, weights_global, weights_local):
    """Shard full inputs into per-core input maps."""
    embedding = np.asarray(embedding, dtype=np.float32)
    index = np.asarray(index)
    weights_global = np.asarray(weights_global, dtype=np.float32)
    weights_local = np.asarray(weights_local, dtype=np.float32)

    embT = np.ascontiguousarray(embedding.T).astype(np.float16)
    emb16 = embedding.astype(np.float16)
    wt = np.ascontiguousarray(weights_global.T)

    # pad cells to NCORES * CLOC
    tot = NCORES * CLOC
    idx_pad = np.zeros((tot, PAIRS, 2), dtype=np.int32)
    idx_pad[:CELLS] = index
    wl_pad = np.zeros((tot, D), dtype=np.float32)
    wl_pad[:CELLS] = weights_local

    in_maps = []
    for k in range(NCORES):
        cells = slice(k * CLOC, (k + 1) * CLOC)
        # x16 on each wl factor => W_cT scaled x256 (keeps fp16 normal range);
        # assemble_output divides the result by 256.
        wl_core = np.ascontiguousarray(wl_pad[cells]) * 16.0  # [CLOC, D]
        idx_core = idx_pad[cells].astype(np.int16)  # [CLOC, PAIRS, 2]

        # wrapped index layout: [16, PAIRS//16] pattern tiled to 128 partitions
        def wrap(a):  # a: [CLOC, PAIRS] -> [128, CLOC, PAIRS//16]
            w = a.reshape(CLOC, PAIRS // 16, 16).transpose(2, 0, 1)
            return np.tile(w, (8, 1, 1))

        arr = np.stack([wrap(idx_core[:, :, 0]), wrap(idx_core[:, :, 1])], axis=1)

        in_maps.append({
            "embT": embT,
            "emb16": emb16,
            "wt": wt,
            "wl": wl_core,
            "wlcol": np.ascontiguousarray(
                wl_core.reshape(CLOC, 4, 128).transpose(2, 0, 1)
            ),
            "idx": np.ascontiguousarray(arr),  # [128, 2, CLOC, PAIRS//16]
        })
    return in_maps


def assemble_output(results):
    """results: list of per-core dicts with 'outv' [CLOC, 128, PAIRS//128]."""
    full = np.empty((NCORES * CLOC, PAIRS), dtype=np.float32)
    for k, res in enumerate(results):
        outv = np.asarray(res["outv"])  # [CLOC, 128, 64]
        full[k * CLOC:(k + 1) * CLOC] = outv.transpose(0, 2, 1).reshape(CLOC, PAIRS)
    full *= 1.0 / 256.0
    return full[:CELLS]


def kernel(embedding, index, weights_global, weights_local):
    nc = get_program()
    in_maps = make_in_maps(embedding, index, weights_global, weights_local)
    res = run_bass_kernel_spmd(nc, in_maps, list(range(NCORES)))
    return assemble_output(res.results)



# revision 9
# speedup vs baseline: 1.0362x; 1.0362x over previous
"""Trainium2 Bass kernel for nn_BilinearDecoder.

Math (per cell c, pair p):
    out[c,p] = sum_{n,k} emb[i0,n] * wl[c,n] * W[n,k] * wl[c,k] * emb[i1,k]

Restructured as:
    That_c[e,n] = wl[c,n] * sum_k W[n,k] * wl[c,k] * emb[e,k]   (matmul over entities)
    out[c,p]   = sum_n emb[i0[c,p],n] * That_c[i1[c,p],n]       (gather + dot)

Sharding: data-parallel over cells. 39 cells -> 8 cores x 5 slots (last slot
of core 7 is padding). Embedding + weights replicated per core.

Per-core pipeline (Tile framework):
  - load embT (fp32, lhsT for matmul), W^T, wl, wrapped int16 gather indices
  - per cell: build W_cT = W^T * wl[k](partition) * wl[n](free)  (vector)
              That = emb @ W_cT  (128 matmuls, fp32, PSUM accumulate)
              cast That -> fp16, store to DRAM scratch
              dma_gather rows of emb16 (A side) and That (B side), fp16
              tensor_mul + tensor_reduce -> out columns

dma_gather layout contracts (HW-validated):
  - indices int16, SBUF tile [128, n/16]: idx j at [j%16, j//16], the 16-row
    pattern replicated 8x down the partitions.
  - output [128, n/128, D]: row j lands at partition j%128, free tile j//128.
Output pair t*128+p therefore sits at out partition p, column t; the host
transposes [CLOC, 128, 64] -> [CLOC, 8192].
"""

import numpy as np
from contextlib import ExitStack

import concourse.bass as bass
import concourse.tile as tile
from concourse import bacc, mybir
from concourse.bass_utils import run_bass_kernel_spmd
from bass_rust import add_dep_helper

CELLS, PAIRS, D, N = 39, 8192, 512, 4096
NCORES, CLOC = 8, 5  # 8 cores x 5 cell slots = 40 >= 39
NCHUNK = 8  # gather chunks per cell
CHUNK = PAIRS // NCHUNK  # 512 pairs per chunk
NSUB = CHUNK // 128  # 4 sub-tiles of 128 pairs per chunk

F32 = mybir.dt.float32
F16 = mybir.dt.float16
I16 = mybir.dt.int16

_PROGRAM = None


def build_program(nchunk=8, gp_bufs=4, zp_bufs=2, tst_bufs=3, psum_bufs=6,
                  op_bufs=CLOC, ag_bufs=7, do_matmul=True, do_gather=True,
                  do_dot=True, reps=1, nqueues=4):
    chunk = PAIRS // nchunk
    nsub = chunk // 128
    nc = bacc.Bacc("TRN2", target_bir_lowering=False, debug=False,
                   num_swdge_queues=nqueues)

    embT = nc.dram_tensor("embT", [D, N], F16, kind="ExternalInput")
    emb16 = nc.dram_tensor("emb16", [N, D], F16, kind="ExternalInput")
    wt = nc.dram_tensor("wt", [D, D], F32, kind="ExternalInput")
    wl = nc.dram_tensor("wl", [CLOC, D], F32, kind="ExternalInput")
    wlcol = nc.dram_tensor("wlcol", [128, CLOC, 4], F32, kind="ExternalInput")
    idx = nc.dram_tensor("idx", [128, 2, CLOC, PAIRS // 16], I16,
                         kind="ExternalInput")
    outv = nc.dram_tensor("outv", [CLOC, 128, PAIRS // 128], F32,
                          kind="ExternalOutput")
    that = nc.dram_tensor("that", [CLOC, N, D], F16)  # internal scratch

    with tile.TileContext(nc) as tc, ExitStack() as ctx:
        singles = ctx.enter_context(tc.tile_pool(name="singles", bufs=1))
        idxp = ctx.enter_context(tc.tile_pool(name="idxp", bufs=2))
        wlp = ctx.enter_context(tc.tile_pool(name="wlp", bufs=2))
        psum = ctx.enter_context(tc.tile_pool(name="psum", bufs=psum_bufs, space="PSUM"))
        zp = ctx.enter_context(tc.tile_pool(name="zp", bufs=zp_bufs))
        tstp = ctx.enter_context(tc.tile_pool(name="tst", bufs=tst_bufs))
        gp = ctx.enter_context(tc.tile_pool(name="gp", bufs=gp_bufs))
        op = ctx.enter_context(tc.tile_pool(name="op", bufs=op_bufs))
        agp = ctx.enter_context(tc.tile_pool(name="agp", bufs=ag_bufs))

        # The rep loop replays the COMPLETE kernel body (input loads, weight
        # prep, compute, output stores) so that test.py's replication
        # differencing measures one full execution, not just the inner loop.
        for rep in range(reps):
            # ---- per-execution loads ----
            embT_t = singles.tile([128, 4, N], F16, tag="embT")
            for kt in range(4):
                nc.sync.dma_start(embT_t[:, kt, :], embT[kt * 128:(kt + 1) * 128, :])
            wt_t = singles.tile([128, 4, D], F32, tag="wt")
            for kt in range(4):
                nc.sync.dma_start(wt_t[:, kt, :], wt[kt * 128:(kt + 1) * 128, :])
            wlcol_t = singles.tile([128, CLOC, 4], F32, tag="wlcol")
            nc.sync.dma_start(wlcol_t, wlcol[:])
            # double-buffered: the reload for rep r+1 must not wait for all of
            # rep r's gathers to release the tile
            idx_t = idxp.tile([128, 2, CLOC, PAIRS // 16], I16, tag="idx")
            nc.sync.dma_start(idx_t, idx[:])

            # ---- build all W_cT up-front (keeps DVE free during dot stage) ----
            wct_t = singles.tile([128, CLOC, 4, D], F16, tag="wct")
            for c in range(CLOC):
                wlr = wlp.tile([128, D], F32)
                nc.scalar.dma_start(wlr, wl[c:c + 1, :].to_broadcast([128, D]))
                for kt in range(4):
                    nc.vector.tensor_mul(wct_t[:, c, kt], wt_t[:, kt], wlr)
                    nc.vector.tensor_scalar_mul(
                        wct_t[:, c, kt], wct_t[:, c, kt], wlcol_t[:, c, kt:kt + 1]
                    )

            def issue_ags(c):
                ags = []
                for ch in range(nchunk if do_gather else 0):
                    ag = agp.tile([128, nsub, D], F16, tag="ag")
                    isl = slice(ch * (chunk // 16), (ch + 1) * (chunk // 16))
                    nc.gpsimd.dma_gather(
                        ag,
                        emb16[:],
                        idx_t[:, 0, c, isl],
                        num_idxs=chunk,
                        num_idxs_reg=chunk,
                        elem_size=D,
                        queue_num=ch % nqueues,
                    )
                    ags.append(ag)
                return ags

            outsbs = []
            for c in range(CLOC):
                that_stores = []
                # ---- That_c = emb @ W_cT : [N, D] fp32 -> fp16 -> DRAM ----
                # 4 et-tiles share one store (bigger DMAs, fewer ops contending
                # with the gather stream)
                tst = None
                for et in range(N // 128 if do_matmul else 0):
                    ps = psum.tile([128, D], F32)
                    for kt in range(4):
                        nc.tensor.matmul(
                            ps,
                            embT_t[:, kt, et * 128:(et + 1) * 128],
                            wct_t[:, c, kt],
                            start=(kt == 0),
                            stop=(kt == 3),
                        )
                    if et % 4 == 0:
                        tst = tstp.tile([128, 4, D], F16)
                    nc.scalar.copy(tst[:, et % 4, :], ps)
                    if et % 4 == 3:
                        st_inst = nc.sync.dma_start(
                            that[c, (et - 3) * 128:(et + 1) * 128, :].rearrange(
                                "(j p) d -> p j d", p=128
                            ),
                            tst,
                        )
                        that_stores.append(st_inst)

                # ---- gather + dot ----
                # All A-side gathers are issued before any That-side gather:
                # the first tg waits on the That stores and would otherwise
                # head-of-line block the independent ag gathers on the
                # in-order Pool engine queue.
                ags = issue_ags(c)
                outsb = op.tile([128, PAIRS // 128], F32)
                if not do_dot:
                    nc.vector.memset(outsb, 0.0)
                for ch in range(nchunk if do_gather else 0):
                    tg = gp.tile([128, nsub, D], F16, tag="tg")
                    isl = slice(ch * (chunk // 16), (ch + 1) * (chunk // 16))
                    tg_inst = nc.gpsimd.dma_gather(
                        tg,
                        that[c] if do_matmul else emb16[:],
                        idx_t[:, 1, c, isl],
                        num_idxs=chunk,
                        num_idxs_reg=chunk,
                        elem_size=D,
                        queue_num=ch % nqueues,
                    )
                    for st_inst in that_stores:
                        add_dep_helper(tg_inst.ins, st_inst.ins,
                                       reason="that DRAM write -> gather read")
                    if do_dot:
                        z = zp.tile([128, nsub, D], F16)
                        nc.vector.tensor_mul(z, ags[ch], tg)
                        nc.vector.tensor_reduce(
                            outsb[:, ch * nsub:(ch + 1) * nsub],
                            z,
                            axis=mybir.AxisListType.X,
                            op=mybir.AluOpType.add,
                        )
                outsbs.append(outsb)

            # outv stores issue from the Activation HWDGE queue: on the SP
            # queue they would head-of-line block the next rep's loads (they
            # wait on the full dot phase)
            for c in range(CLOC):
                nc.scalar.dma_start(outv[c], outsbs[-CLOC + c])

    nc.compile()
    return nc


def get_program():
    global _PROGRAM
    if _PROGRAM is None:
        _PROGRAM = build_program()
    return _PROGRAM


def make_in_maps(embedding, index, weights_global, weights_local):
    """Shard full inputs into per-core input maps."""
    embedding = np.asarray(embedding, dtype=np.float32)
    index = np.asarray(index)
    weights_global = np.asarray(weights_global, dtype=np.float32)
    weights_local = np.asarray(weights_local, dtype=np.float32)

    embT = np.ascontiguousarray(embedding.T).astype(np.float16)
    emb16 = embedding.astype(np.float16)
    wt = np.ascontiguousarray(weights_global.T)

    # pad cells to NCORES * CLOC
    tot = NCORES * CLOC
    idx_pad = np.zeros((tot, PAIRS, 2), dtype=np.int32)
    idx_pad[:CELLS] = index
    wl_pad = np.zeros((tot, D), dtype=np.float32)
    wl_pad[:CELLS] = weights_local

    in_maps = []
    for k in range(NCORES):
        cells = slice(k * CLOC, (k + 1) * CLOC)
        # x16 on each wl factor => W_cT scaled x256 (keeps fp16 normal range);
        # assemble_output divides the result by 256.
        wl_core = np.ascontiguousarray(wl_pad[cells]) * 16.0  # [CLOC, D]
        idx_core = idx_pad[cells].astype(np.int16)  # [CLOC, PAIRS, 2]

        # wrapped index layout: [16, PAIRS//16] pattern tiled to 128 partitions
        def wrap(a):  # a: [CLOC, PAIRS] -> [128, CLOC, PAIRS//16]
            w = a.reshape(CLOC, PAIRS // 16, 16).transpose(2, 0, 1)
            return np.tile(w, (8, 1, 1))

        arr = np.stack([wrap(idx_core[:, :, 0]), wrap(idx_core[:, :, 1])], axis=1)

        in_maps.append({
            "embT": embT,
            "emb16": emb16,
            "wt": wt,
            "wl": wl_core,
            "wlcol": np.ascontiguousarray(
                wl_core.reshape(CLOC, 4, 128).transpose(2, 0, 1)
            ),
            "idx": np.ascontiguousarray(arr),  # [128, 2, CLOC, PAIRS//16]
        })
    return in_maps


def assemble_output(results):
    """results: list of per-core dicts with 'outv' [CLOC, 128, PAIRS//128]."""
    full = np.empty((NCORES * CLOC, PAIRS), dtype=np.float32)
    for k, res in enumerate(results):
        outv = np.asarray(res["outv"])  # [CLOC, 128, 64]
        full[k * CLOC:(k + 1) * CLOC] = outv.transpose(0, 2, 1).reshape(CLOC, PAIRS)
    full *= 1.0 / 256.0
    return full[:CELLS]


def kernel(embedding, index, weights_global, weights_local):
    nc = get_program()
    in_maps = make_in_maps(embedding, index, weights_global, weights_local)
    res = run_bass_kernel_spmd(nc, in_maps, list(range(NCORES)))
    return assemble_output(res.results)

